# revision 3
# baseline (speedup 1.0000x reference)
"""Trainium2 Bass kernel for masked (sparse) multi-head attention, v2.1.

Reference (per batch): qkv = x @ w_qkv.T; q *= D**-0.5; s = q@k.T per head;
e = exp(s) * ap  (ap = key policy, self-attend always allowed);
attn = e / sum_m e; y = (attn @ v) @ w_proj.T + b_proj (b_proj on host).

Sharding: data parallel, batch b -> core b (B == n_cores == 8).

Design (CoreSim cost model: a matmul costs out-free-size rows x 0.4167ns,
independent of K and M; ACT activations cost ~0.833ns/elem/partition):
  - P@v in NATURAL orientation (queries on psum partitions, head dim on
    the free axis): 12h*8jn*6jm*65 rows instead of 12h*6jm*1024.  P =
    exp(ST) feeds it directly as lhsT; v blocks carry a ones column at
    col D so each (head, qchunk) psum accumulates its softmax
    denominator; normalization is a per-partition DVE scalar multiply.
  - normalized o (fp16) is PE-transposed per [128,128] block; projection
    contracts K=128 stacked c-chunks (8jn*6cc*768 rows, vs 12h*8*768 at
    K=64 before).  The projection is SPLIT: cc 0..3 (heads 0..7) run
    while later heads' exps still stream, folded to SBUF; cc 4..5 +
    a DVE/Pool add finish after the last exp.
  - P / v / diag are bf16 (exp(s) reaches ~1e6; fp16 would overflow);
    qT/kT and the transpose/projection pipeline are fp16.
  - single psum pool, tags S and B ([128,1024] x 2 bufs each = 8 banks).
    S ring: warmup, score S tiles, proj y tiles.  B ring: qk psums,
    v psums, gm psums, P@v O tiles, transpose tiles.
  - emission merges the bulk stream (qk, v, gm) with exp-paced score
    pairs, P@v heads, and pre-projection groups so the ACT engine (72
    exps ~ 75us) is the only near-critical chain and PE never
    head-of-line blocks.
  - separate q-half / k-half weight tiles (a shared tile's second DMA
    writer falsely serialized the first matmul); ~15 warmup matmuls on a
    memset tile cover the PE p-state ramp while the first DMAs land.
"""

import sys

import numpy as np

sys.path.insert(0, "/opt/trn_rl_repo")

from contextlib import ExitStack

import concourse.bass as bass
import concourse.tile as tile
from concourse import mybir
from concourse.bacc import Bacc

F32 = mybir.dt.float32
F32R = mybir.dt.float32r
BF16 = mybir.dt.bfloat16
FP16 = mybir.dt.float16
AF = mybir.ActivationFunctionType

B, N, C, H = 8, 1024, 768, 12
D = C // H             # 64
SCALE = D ** -0.5
CH = C // 128           # 6 c-chunks (2 heads each)
NJ = N // 128           # 8 n-chunks
MJ = N // 128           # 8 m-chunks
NEG = -10000.0          # exp(s + NEG) == 0.0 for any realistic s
W = D + 1               # per-head v block width; ones col at D
NWARM = 12              # PE p-state warmup matmuls
PACE_ROWS = 1520        # bulk rows between score pairs; a pair itself is
                        # 1024 rows, so S-tile spacing ~= ACT exp cadence
P_BUFS = 9              # P-tile backlog (heads in flight)
CPRE = 4                # proj cc-chunks folded early (heads 0..2*CPRE-1)


def build_nc(mk: int, jd: int) -> bass.Bass:
    """mk = chunks holding all kept tokens; jd = first chunk with any
    dropped token (diag machinery only needed for chunks >= jd)."""
    nc = Bacc()

    xT = nc.declare_dram_parameter("xT", [C, N], FP16, isOutput=False)
    wqkvT = nc.declare_dram_parameter("wqkvT", [C, 3 * C], FP16, isOutput=False)
    wprojT = nc.declare_dram_parameter("wprojT", [C, C], FP16, isOutput=False)
    cpackA = nc.declare_dram_parameter("cpackA", [128, 2 * MJ], F32,
                                       isOutput=False)
    eheadB = nc.declare_dram_parameter("eheadB", [128, CH * H], FP16,
                                       isOutput=False)
    identH = nc.declare_dram_parameter("identH", [128, 128], FP16,
                                       isOutput=False)
    identB = nc.declare_dram_parameter("identB", [128, 128], BF16,
                                       isOutput=False)
    y = nc.declare_dram_parameter("y", [N, C], F32, isOutput=True)

    with ExitStack() as ctx:
        tc = ctx.enter_context(tile.TileContext(nc))

        consts = ctx.enter_context(tc.tile_pool(name="consts", bufs=1))
        qk_pool = ctx.enter_context(tc.tile_pool(name="qk", bufs=1))
        v_pool = ctx.enter_context(tc.tile_pool(name="v", bufs=1))
        wp_pool = ctx.enter_context(tc.tile_pool(name="wpp", bufs=1))
        p_pool = ctx.enter_context(tc.tile_pool(name="pp", bufs=P_BUFS))
        oN_pool = ctx.enter_context(tc.tile_pool(name="oN", bufs=1))
        rec_pool = ctx.enter_context(tc.tile_pool(name="rec", bufs=2))
        dg_pool = ctx.enter_context(tc.tile_pool(name="dg", bufs=4))
        pp = ctx.enter_context(tc.tile_pool(name="psum", bufs=2, space="PSUM"))

        # ---- constants --------------------------------------------------
        cpa_sb = consts.tile([128, 2 * MJ], F32, tag="cpa", name="cpa")
        lm_sb = cpa_sb[:, 0:MJ]
        omp_sb = cpa_sb[:, MJ:2 * MJ]
        eh_sb = consts.tile([128, CH * H], FP16, tag="eh", name="eh")
        id16_sb = consts.tile([128, 128], FP16, tag="id16", name="id16")
        idb_sb = consts.tile([128, 128], BF16, tag="idb", name="idb")
        dummy = consts.tile([128, 128], FP16, tag="dummy", name="dummy")
        gm_sb = consts.tile([128, MJ, H], F32, tag="gm", name="gm")

        # persistent activation tiles
        qT = [qk_pool.tile([128, N], FP16, tag=f"qT{cc}", name=f"qT{cc}")
              for cc in range(CH)]
        kT = [qk_pool.tile([128, N], FP16, tag=f"kT{cc}", name=f"kT{cc}")
              for cc in range(CH)]
        v16 = [v_pool.tile([128, H, W], BF16, tag=f"v{j}", name=f"v{j}")
               for j in range(NJ)]
        wp_sb = [wp_pool.tile([128, C], FP16, tag=f"wp{cc}", name=f"wp{cc}")
                 for cc in range(CH)]
        oN = [oN_pool.tile([128, C], FP16, tag=f"oN{j}", name=f"oN{j}")
              for j in range(NJ)]
        # oT / ypre / yout pools are created after the ph1 input pool
        # closes so their SBUF reuses the freed xT/w space (stack alloc).
        oT = []
        ypre = []
        yout_pool_ref = []

        # ---- warmup: cover the PE p-state ramp before DMAs land ---------
        nc.vector.memset(dummy[:], 0.0)
        for i in range(NWARM):
            wt = pp.tile([128, 1024], F32, tag="S", name="warm")
            nc.tensor.matmul(wt[:, 0:128], dummy[:], dummy[:],
                             start=True, stop=True)

        # ---- v16 ones columns (DVE idle early) --------------------------
        for j in range(NJ):
            nc.vector.memset(v16[j][:, :, D:W], 1.0)

        P_tiles = [None] * H

        def emit_score_pair(h, jm):
            """ST chunk jm for head h: 2 matmuls + ACT exp -> P bf16."""
            cc, off = divmod(h, 2)
            off *= D
            if P_tiles[h] is None:
                P_tiles[h] = p_pool.tile([128, mk, N], BF16, tag="P",
                                         name=f"P{h}")
            S = pp.tile([128, N], F32, tag="S", name="S")
            for nn in range(2):
                nc.tensor.matmul(
                    S[:, nn * 512:(nn + 1) * 512],
                    kT[cc][off:off + D, jm * 128:(jm + 1) * 128],
                    qT[cc][off:off + D, nn * 512:(nn + 1) * 512],
                    start=True, stop=True)
            nc.scalar.activation(P_tiles[h][:, jm, :], S[:], AF.Exp,
                                 bias=lm_sb[:, jm:jm + 1])

        dg_flip = [True]

        def emit_pv_head(h):
            """P@v for head h, natural orientation, 8 q-chunk groups.
            One batched reciprocal per head; normalize muls alternate
            DVE/Pool so the O-tile drain keeps up with the PE."""
            O = pp.tile([128, MJ, 128], F32, tag="B", name=f"O{h}")
            rec = rec_pool.tile([128, MJ, 1], F32, tag="rec", name="rec")
            for jn in range(MJ):
                has_diag = jn >= jd
                for jm in range(mk):
                    nc.tensor.matmul(
                        O[:, jn, 0:W],
                        P_tiles[h][:, jm, jn * 128:(jn + 1) * 128],
                        v16[jm][:, h, :],
                        start=(jm == 0),
                        stop=(jm == mk - 1 and not has_diag))
                if has_diag:
                    dg = dg_pool.tile([128, 128], BF16, tag="dg", name="dg")
                    eng = nc.vector if dg_flip[0] else nc.gpsimd
                    dg_flip[0] = not dg_flip[0]
                    eng.tensor_scalar_mul(dg[:], idb_sb[:],
                                          gm_sb[:, jn, h:h + 1])
                    nc.tensor.matmul(O[:, jn, 0:W], dg[:],
                                     v16[jn][:, h, :],
                                     start=False, stop=True)
            nc.vector.reciprocal(rec[:], O[:, :, D:W])
            with nc.allow_low_precision(reason="fp16 o norm"):
                for jn in range(MJ):
                    eng = nc.vector if jn % 2 == 0 else nc.gpsimd
                    eng.tensor_scalar_mul(
                        oN[jn][:, h * D:(h + 1) * D],
                        O[:, jn, 0:D], rec[:, jn, :])

        def emit_pre_tp(jn):
            """Transpose cc 0..CPRE-1 of oN[jn]; DVE drains to oT."""
            tp = pp.tile([128, CPRE, 128], FP16, tag="B", name=f"tp{jn}")
            for cc in range(CPRE):
                nc.tensor.matmul(tp[:, cc, :],
                                 oN[jn][:, cc * 128:(cc + 1) * 128],
                                 id16_sb[:], is_transpose=True,
                                 start=True, stop=True)
            for cc in range(CPRE):
                nc.vector.tensor_copy(oT[cc][:, jn * 128:(jn + 1) * 128],
                                      tp[:, cc, :])

        def emit_pre_proj(jn):
            """Fold the first CPRE proj chunks into ypre[jn].  Uses the
            S-ring: the score stream is finished by the time these run."""
            yps = pp.tile([128, 1024], F32, tag="S", name="yps")
            for sl0, sl1 in ((0, 512), (512, C)):
                for cc in range(CPRE):
                    nc.tensor.matmul(
                        yps[:, sl0:sl1],
                        oT[cc][:, jn * 128:(jn + 1) * 128],
                        wp_sb[cc][:, sl0:sl1],
                        start=(cc == 0), stop=(cc == CPRE - 1))
            with nc.allow_low_precision(reason="fp16 ypre fold"):
                eng = nc.gpsimd if jn % 2 == 0 else nc.vector
                eng.tensor_copy(ypre[jn][:], yps[:, 0:C])

        def emit_tail_tp(jn):
            """Transpose cc CPRE..5 of oN[jn]; ACT drains to oT."""
            tp = pp.tile([128, CH - CPRE, 128], FP16, tag="B",
                         name=f"tt{jn}")
            for i, cc in enumerate(range(CPRE, CH)):
                nc.tensor.matmul(tp[:, i, :],
                                 oN[jn][:, cc * 128:(cc + 1) * 128],
                                 id16_sb[:], is_transpose=True,
                                 start=True, stop=True)
            for i, cc in enumerate(range(CPRE, CH)):
                if jn % 2 == 0:
                    nc.scalar.copy(oT[cc][:, jn * 128:(jn + 1) * 128],
                                   tp[:, i, :])
                else:
                    nc.vector.tensor_copy(
                        oT[cc][:, jn * 128:(jn + 1) * 128], tp[:, i, :])

        def emit_tail_proj(jn):
            """Project cc CPRE..5, add ypre, copy out, DMA."""
            yps = pp.tile([128, 1024], F32, tag="S", name="yts")
            for sl0, sl1 in ((0, 512), (512, C)):
                for i, cc in enumerate(range(CPRE, CH)):
                    nc.tensor.matmul(
                        yps[:, sl0:sl1],
                        oT[cc][:, jn * 128:(jn + 1) * 128],
                        wp_sb[cc][:, sl0:sl1],
                        start=(cc == CPRE), stop=(cc == CH - 1))
            ysb = yout_pool_ref[0].tile([128, C], F32, tag="ysb",
                                        name="ysb")
            with nc.allow_low_precision(reason="f32 add"):
                nc.vector.tensor_add(ysb[:, 0:512], ypre[jn][:, 0:512],
                                     yps[:, 0:512])
                nc.gpsimd.tensor_add(ysb[:, 512:C], ypre[jn][:, 512:C],
                                     yps[:, 512:C])
            nc.sync.dma_start(out=y[jn * 128:(jn + 1) * 128, 0:512],
                              in_=ysb[:, 0:512])
            nc.gpsimd.dma_start(out=y[jn * 128:(jn + 1) * 128, 512:C],
                                in_=ysb[:, 512:C])

        # ================= phase A/B: qkv + scores ======================
        with tc.tile_pool(name="ph1", bufs=1) as ph1:
            xT_sb = [ph1.tile([128, N], FP16, tag=f"xT{kk}", name=f"xs{kk}")
                     for kk in range(CH)]
            wqq_sb = [ph1.tile([128, C], FP16, tag=f"wq{kk}", name=f"wq{kk}")
                      for kk in range(CH)]
            # NOTE: wqq shares ph1's lifetime; P_BUFS budget counts on the
            # post-ph1 release for the oT/ypre/yout pools.
            wqk_sb = [ph1.tile([128, C], FP16, tag=f"wk{kk}", name=f"wk{kk}")
                      for kk in range(CH)]
            wv_sb = [ph1.tile([128, C], FP16, tag=f"wv{kk}", name=f"wv{kk}")
                     for kk in range(CH)]

            # ---- input DMAs (order on each ring == arrival order) -------
            for kk in range(CH):
                deng = nc.sync if kk % 2 == 0 else nc.gpsimd
                deng.dma_start(out=xT_sb[kk][:],
                               in_=xT[kk * 128:(kk + 1) * 128, :])
                deng.dma_start(out=wqq_sb[kk][:],
                               in_=wqkvT[kk * 128:(kk + 1) * 128, 0:C])
                if kk == 1:
                    nc.gpsimd.dma_start(out=cpa_sb[:], in_=cpackA[:, :])
            nc.gpsimd.dma_start(out=eh_sb[:], in_=eheadB[:, :])
            nc.gpsimd.dma_start(out=id16_sb[:], in_=identH[:, :])
            nc.gpsimd.dma_start(out=idb_sb[:], in_=identB[:, :])
            for kk in range(CH):
                deng = nc.sync if kk % 2 == 0 else nc.gpsimd
                deng.dma_start(out=wqk_sb[kk][:],
                               in_=wqkvT[kk * 128:(kk + 1) * 128, C:2 * C])
            for kk in range(CH):
                deng = nc.sync if kk % 2 == 0 else nc.gpsimd
                deng.dma_start(out=wv_sb[kk][:],
                               in_=wqkvT[kk * 128:(kk + 1) * 128, 2 * C:3 * C])
            for cc in range(CH):
                deng = nc.gpsimd if cc % 2 == 0 else nc.sync
                deng.dma_start(out=wp_sb[cc][:],
                               in_=wprojT[cc * 128:(cc + 1) * 128, :])

            def gen_qk(which, cc, split=False):
                """q or k chunk cc: 12 matmuls, then fp16 copy. Yields rows
                after each matmul so score pairs can interleave.  With
                split=True the kk contraction runs as two 3-chunk psum
                groups combined by a DVE add, so the first group only
                depends on the first 3 input DMAs (group deps are hoisted
                to the group's first matmul)."""
                wsrc = wqq_sb if which == "q" else wqk_sb
                dst = (qT if which == "q" else kT)[cc]
                ps = pp.tile([128, N], F32, tag="B", name=f"{which}{cc}")
                if split:
                    ps2 = pp.tile([128, N], F32, tag="S", name=f"{which}{cc}b")
                    for half, (k0, k1) in enumerate(((0, 3), (3, CH))):
                        dstp = ps if half == 0 else ps2
                        for nn in range(2):
                            for kk in range(k0, k1):
                                nc.tensor.matmul(
                                    dstp[:, nn * 512:(nn + 1) * 512],
                                    wsrc[kk][:, cc * 128:(cc + 1) * 128],
                                    xT_sb[kk][:, nn * 512:(nn + 1) * 512],
                                    start=(kk == k0), stop=(kk == k1 - 1))
                                if half == 1 and nn == 1 and kk == k1 - 1:
                                    with nc.allow_low_precision(
                                            reason="fp16 qk combine"):
                                        nc.vector.tensor_copy(dst[:], ps2[:])
                                        nc.vector.tensor_add(dst[:], dst[:],
                                                             ps[:])
                                yield 512
                    return
                for nn in range(2):
                    for kk in range(CH):
                        nc.tensor.matmul(
                            ps[:, nn * 512:(nn + 1) * 512],
                            wsrc[kk][:, cc * 128:(cc + 1) * 128],
                            xT_sb[kk][:, nn * 512:(nn + 1) * 512],
                            start=(kk == 0), stop=(kk == CH - 1))
                        if nn == 1 and kk == CH - 1:
                            nc.vector.tensor_copy(dst[:], ps[:])
                        yield 512

            def gen_v(jn):
                """v chunk jn: 12 matmuls, then strided bf16 copy."""
                ps = pp.tile([128, 1024], F32, tag="B", name=f"vp{jn}")
                for sl0, sl1 in ((0, 512), (512, C)):
                    for kk in range(CH):
                        nc.tensor.matmul(
                            ps[:, sl0:sl1],
                            xT_sb[kk][:, jn * 128:(jn + 1) * 128],
                            wv_sb[kk][:, sl0:sl1],
                            start=(kk == 0), stop=(kk == CH - 1))
                        if sl1 == C and kk == CH - 1:
                            ps3 = ps[:, 0:C].rearrange("p (h d) -> p h d", h=H)
                            nc.vector.tensor_copy(v16[jn][:, :, 0:D], ps3)
                        yield sl1 - sl0

            def gen_gm(prod_pool):
                """diag self-term magnitudes for chunks >= jd.  prod
                reuses the dead wqq weight tiles (last read: q-chunk 5,
                which precedes gm in emission order) via bitcast."""
                prod = []
                nd = MJ - jd
                for cc in range(CH):
                    pr = wqq_sb[cc][:, 0:nd * 128]
                    eng = nc.gpsimd if cc % 2 == 0 else nc.vector
                    with nc.allow_low_precision(reason="fp16 prod"):
                        eng.tensor_mul(pr, qT[cc][:, jd * 128:],
                                       kT[cc][:, jd * 128:])
                    prod.append(pr)
                for jm in range(jd, MJ):
                    gps = pp.tile([128, 1024], F32, tag="B", name="gps")
                    for cc in range(CH):
                        nc.tensor.matmul(
                            gps[:, 0:H],
                            prod[cc][:, (jm - jd) * 128:(jm - jd + 1) * 128],
                            eh_sb[:, cc * H:(cc + 1) * H],
                            start=(cc == 0), stop=(cc == CH - 1))
                        yield H
                    nc.scalar.activation(gm_sb[:, jm, :], gps[:, 0:H], AF.Exp)
                    nc.vector.tensor_scalar_mul(gm_sb[:, jm, :],
                                                gm_sb[:, jm, :],
                                                omp_sb[:, jm:jm + 1])

            def bulk_stream(prod_pool):
                # q0,q1,k0,q2,k1,... : k_cc as early as its wk DMAs allow,
                # so the exp stream starts ~10us in
                yield from gen_qk("q", 0, split=True)
                yield from gen_qk("q", 1)
                yield from gen_qk("k", 0, split=True)
                state["qk_pairs"] = 1
                for cc in range(2, CH):
                    yield from gen_qk("q", cc)
                    yield from gen_qk("k", cc - 1)
                    state["qk_pairs"] = cc
                yield from gen_qk("k", CH - 1)
                state["qk_pairs"] = CH
                yield from gen_gm(prod_pool)
                for jn in range(NJ):
                    yield from gen_v(jn)

            score_list = [(h, jm) for h in range(H) for jm in range(mk)]
            state = {"si": 0, "credit": 0.0, "qk_pairs": 0, "pv": 0}

            def score_eligible():
                if state["si"] >= len(score_list):
                    return False
                h, _ = score_list[state["si"]]
                if (h // 2) >= state["qk_pairs"]:
                    return False
                # P-ring: the exp for head h allocates P slot h % P_BUFS,
                # which frees only when P@v of head h-P_BUFS is done.
                # Emitting the pair earlier jams the S-ring behind it.
                return h < P_BUFS or state["pv"] >= h - P_BUFS + 1

            def pump_scores():
                while state["credit"] >= PACE_ROWS and score_eligible():
                    emit_score_pair(*score_list[state["si"]])
                    state["si"] += 1
                    state["credit"] -= PACE_ROWS
                # no banking: a credit surplus would burst pairs back-to-back
                # and the S-ring (2 slots, ACT-paced) head-of-line blocks
                # everything emitted after them
                state["credit"] = min(state["credit"], 1.2 * PACE_ROWS)

            for rows in bulk_stream(None):
                state["credit"] += rows
                pump_scores()

        # ============ phase C: P@v + pre-projection + tail ==============
        oT_pool = ctx.enter_context(tc.tile_pool(name="oTp", bufs=1))
        ypre_pool = ctx.enter_context(tc.tile_pool(name="ypre", bufs=1))
        yout_pool_ref.append(
            ctx.enter_context(tc.tile_pool(name="yout", bufs=2)))
        oT.extend(oT_pool.tile([128, N], FP16, tag=f"oT{cc}", name=f"oT{cc}")
                  for cc in range(CH))
        ypre.extend(ypre_pool.tile([128, C], FP16, tag=f"yp{j}",
                                   name=f"yp{j}") for j in range(NJ))
        # pv h0..h7 run as soon as their exps/psum allow; deferred score
        # pairs (P-ring gated) are pumped between them.
        for h in range(8):
            emit_pv_head(h)
            state["pv"] = h + 1
            while score_eligible():
                emit_score_pair(*score_list[state["si"]])
                state["si"] += 1
        while score_eligible():
            emit_score_pair(*score_list[state["si"]])
            state["si"] += 1
        # pv h8..h11 execute gated on their exps (~6us apart): distribute
        # the pre-projection units into those gaps, tp/proj pipelined.
        pre_units = []
        pre_units.append(("tp", 0))
        for jn in range(NJ):
            if jn + 1 < NJ:
                pre_units.append(("tp", jn + 1))
            pre_units.append(("proj", jn))
        per_gap = (len(pre_units) + 3) // 4
        ui = 0
        for h in range(8, H):
            emit_pv_head(h)
            state["pv"] = h + 1
            while score_eligible():
                emit_score_pair(*score_list[state["si"]])
                state["si"] += 1
            for _ in range(per_gap):
                if ui < len(pre_units):
                    kind, jn = pre_units[ui]
                    (emit_pre_tp if kind == "tp" else emit_pre_proj)(jn)
                    ui += 1
        while ui < len(pre_units):
            kind, jn = pre_units[ui]
            (emit_pre_tp if kind == "tp" else emit_pre_proj)(jn)
            ui += 1
        emit_tail_tp(0)
        emit_tail_tp(1)
        for jn in range(NJ):
            emit_tail_proj(jn)
            if jn + 2 < NJ:
                emit_tail_tp(jn + 2)

    nc.finalize()
    return nc


_NC_CACHE = {}


def _get_nc(mk: int = MJ, jd: int = 0):
    if (mk, jd) not in _NC_CACHE:
        _NC_CACHE[(mk, jd)] = build_nc(mk, jd)
    return _NC_CACHE[(mk, jd)]


def _to_bf16(a):
    import ml_dtypes
    return np.asarray(a, np.float32).astype(ml_dtypes.bfloat16)


def _host_inputs(x, policy, w_qkv, w_proj, b_proj):
    """Shard + permute (kept tokens first) + layout transforms.

    Returns (in_maps, perms, mk, jd)."""
    wqkv_s = np.array(w_qkv, dtype=np.float32, copy=True)
    wqkv_s[0:C] *= np.float32(SCALE)
    wqkvT = np.ascontiguousarray(wqkv_s.T).astype(np.float16)   # [C, 3C]
    wprojT = np.ascontiguousarray(
        np.asarray(w_proj, np.float32).T).astype(np.float16)

    E = np.zeros((C, H), np.float32)
    for c in range(C):
        E[c, c // D] = 1.0
    eheadB = np.ascontiguousarray(
        E.reshape(CH, 128, H).transpose(1, 0, 2).reshape(128, CH * H)
    ).astype(np.float16)
    ident = np.eye(128, dtype=np.float32)
    identH = ident.astype(np.float16)
    identB = _to_bf16(ident)

    in_maps = []
    perms = []
    mk = 1
    jd = MJ - 1
    for b in range(B):
        pol = np.asarray(policy[b], np.float32).reshape(N)
        kept = np.nonzero(pol > 0.5)[0]
        drop = np.nonzero(pol <= 0.5)[0]
        perm = np.concatenate([kept, drop])
        perms.append(perm)
        mk = max(mk, (len(kept) + 127) // 128)
        jd = min(jd, len(kept) // 128)

        xb = np.asarray(x[b], np.float32)[perm, :]          # permuted tokens
        xTb = np.ascontiguousarray(xb.T).astype(np.float16)  # [C, N]
        polp = pol[perm]
        lm = np.where(polp > 0.5, 0.0, NEG).astype(np.float32)
        lm = np.ascontiguousarray(lm.reshape(MJ, 128).T)    # [128, MJ]
        om = np.ascontiguousarray((1.0 - polp).reshape(MJ, 128).T)
        cpackA = np.ascontiguousarray(np.concatenate(
            [lm, om.astype(np.float32)], axis=1))
        in_maps.append({
            "xT": xTb, "wqkvT": wqkvT, "wprojT": wprojT,
            "cpackA": cpackA, "eheadB": eheadB,
            "identH": identH, "identB": identB,
        })
    return in_maps, perms, mk, jd


def kernel(x, policy, w_qkv, w_proj, b_proj):
    from concourse.bass_utils import run_bass_kernel_spmd

    x = np.asarray(x, np.float32)
    policy = np.asarray(policy, np.float32)
    w_qkv = np.asarray(w_qkv, np.float32)
    w_proj = np.asarray(w_proj, np.float32)
    b_proj = np.asarray(b_proj, np.float32)
    in_maps, perms, mk, jd = _host_inputs(x, policy, w_qkv, w_proj, b_proj)
    nc = _get_nc(mk, jd)
    res = run_bass_kernel_spmd(nc, in_maps, list(range(B)))
    out = np.empty((B, N, C), np.float32)
    bp = np.asarray(b_proj, np.float32).reshape(1, C)
    for b in range(B):
        out[b][perms[b]] = res.results[b]["y"] + bp
    return out


# revision 4
# speedup vs baseline: 1.1078x; 1.1078x over previous
"""Trainium2 Bass kernel for masked (sparse) multi-head attention, v2.1.

Reference (per batch): qkv = x @ w_qkv.T; q *= D**-0.5; s = q@k.T per head;
e = exp(s) * ap  (ap = key policy, self-attend always allowed);
attn = e / sum_m e; y = (attn @ v) @ w_proj.T + b_proj (b_proj on host).

Sharding: data parallel, batch b -> core b (B == n_cores == 8).

Design (CoreSim cost model: a matmul costs out-free-size rows x 0.4167ns,
independent of K and M; ACT activations cost ~0.833ns/elem/partition):
  - P@v in NATURAL orientation (queries on psum partitions, head dim on
    the free axis): 12h*8jn*6jm*65 rows instead of 12h*6jm*1024.  P =
    exp(ST) feeds it directly as lhsT; v blocks carry a ones column at
    col D so each (head, qchunk) psum accumulates its softmax
    denominator; normalization is a per-partition DVE scalar multiply.
  - normalized o (fp16) is PE-transposed per [128,128] block; projection
    contracts K=128 stacked c-chunks (8jn*6cc*768 rows, vs 12h*8*768 at
    K=64 before).  The projection is SPLIT: cc 0..3 (heads 0..7) run
    while later heads' exps still stream, folded to SBUF; cc 4..5 +
    a DVE/Pool add finish after the last exp.
  - P / v / diag are bf16 (exp(s) reaches ~1e6; fp16 would overflow);
    qT/kT and the transpose/projection pipeline are fp16.
  - single psum pool, tags S and B ([128,1024] x 2 bufs each = 8 banks).
    S ring: warmup, score S tiles, proj y tiles.  B ring: qk psums,
    v psums, gm psums, P@v O tiles, transpose tiles.
  - emission merges the bulk stream (qk, v, gm) with exp-paced score
    pairs, P@v heads, and pre-projection groups so the ACT engine (72
    exps ~ 75us) is the only near-critical chain and PE never
    head-of-line blocks.
  - separate q-half / k-half weight tiles (a shared tile's second DMA
    writer falsely serialized the first matmul); ~15 warmup matmuls on a
    memset tile cover the PE p-state ramp while the first DMAs land.
"""

import sys

import numpy as np

sys.path.insert(0, "/opt/trn_rl_repo")

from contextlib import ExitStack

import concourse.bass as bass
import concourse.tile as tile
from concourse import mybir
from concourse.bacc import Bacc

F32 = mybir.dt.float32
F32R = mybir.dt.float32r
BF16 = mybir.dt.bfloat16
FP16 = mybir.dt.float16
AF = mybir.ActivationFunctionType

B, N, C, H = 8, 1024, 768, 12
D = C // H             # 64
SCALE = D ** -0.5
CH = C // 128           # 6 c-chunks (2 heads each)
NJ = N // 128           # 8 n-chunks
MJ = N // 128           # 8 m-chunks
NEG = -10000.0          # exp(s + NEG) == 0.0 for any realistic s
W = D + 1               # per-head v block width; ones col at D
NWARM = 12              # PE p-state warmup matmuls
PACE_ROWS = 1520        # bulk rows between score pairs; a pair itself is
                        # 1024 rows, so S-tile spacing ~= ACT exp cadence
P_BUFS = 9              # P-tile backlog (heads in flight)
CPRE = 4                # proj cc-chunks folded early (heads 0..2*CPRE-1)


def build_nc(mk: int, jd: int) -> bass.Bass:
    """mk = chunks holding all kept tokens; jd = first chunk with any
    dropped token (diag machinery only needed for chunks >= jd)."""
    nc = Bacc()

    xT = nc.declare_dram_parameter("xT", [C, N], FP16, isOutput=False)
    wqkvT = nc.declare_dram_parameter("wqkvT", [C, 3 * C], FP16, isOutput=False)
    wprojT = nc.declare_dram_parameter("wprojT", [C, C], FP16, isOutput=False)
    cpackA = nc.declare_dram_parameter("cpackA", [128, 2 * MJ], F32,
                                       isOutput=False)
    eheadB = nc.declare_dram_parameter("eheadB", [128, CH * H], FP16,
                                       isOutput=False)
    identH = nc.declare_dram_parameter("identH", [128, 128], FP16,
                                       isOutput=False)
    identB = nc.declare_dram_parameter("identB", [128, 128], BF16,
                                       isOutput=False)
    y = nc.declare_dram_parameter("y", [N, C], F32, isOutput=True)

    with ExitStack() as ctx:
        tc = ctx.enter_context(tile.TileContext(nc))

        consts = ctx.enter_context(tc.tile_pool(name="consts", bufs=1))
        qk_pool = ctx.enter_context(tc.tile_pool(name="qk", bufs=1))
        v_pool = ctx.enter_context(tc.tile_pool(name="v", bufs=1))
        wp_pool = ctx.enter_context(tc.tile_pool(name="wpp", bufs=1))
        p_pool = ctx.enter_context(tc.tile_pool(name="pp", bufs=P_BUFS))
        oN_pool = ctx.enter_context(tc.tile_pool(name="oN", bufs=1))
        rec_pool = ctx.enter_context(tc.tile_pool(name="rec", bufs=2))
        dg_pool = ctx.enter_context(tc.tile_pool(name="dg", bufs=4))
        pp = ctx.enter_context(tc.tile_pool(name="psum", bufs=2, space="PSUM"))

        # ---- constants --------------------------------------------------
        cpa_sb = consts.tile([128, 2 * MJ], F32, tag="cpa", name="cpa")
        lm_sb = cpa_sb[:, 0:MJ]
        omp_sb = cpa_sb[:, MJ:2 * MJ]
        eh_sb = consts.tile([128, CH * H], FP16, tag="eh", name="eh")
        id16_sb = consts.tile([128, 128], FP16, tag="id16", name="id16")
        idb_sb = consts.tile([128, 128], BF16, tag="idb", name="idb")
        dummy = consts.tile([128, 128], FP16, tag="dummy", name="dummy")
        gm_sb = consts.tile([128, MJ, H], F32, tag="gm", name="gm")

        # persistent activation tiles
        qT = [qk_pool.tile([128, N], FP16, tag=f"qT{cc}", name=f"qT{cc}")
              for cc in range(CH)]
        kT = [qk_pool.tile([128, N], FP16, tag=f"kT{cc}", name=f"kT{cc}")
              for cc in range(CH)]
        v16 = [v_pool.tile([128, H, W], BF16, tag=f"v{j}", name=f"v{j}")
               for j in range(NJ)]
        wp_sb = [wp_pool.tile([128, C], FP16, tag=f"wp{cc}", name=f"wp{cc}")
                 for cc in range(CH)]
        oN = [oN_pool.tile([128, C], FP16, tag=f"oN{j}", name=f"oN{j}")
              for j in range(NJ)]
        # oT / ypre / yout pools are created after the ph1 input pool
        # closes so their SBUF reuses the freed xT/w space (stack alloc).
        oT = []
        ypre = []
        yout_pool_ref = []

        # ---- warmup: cover the PE p-state ramp before DMAs land ---------
        nc.vector.memset(dummy[:], 0.0)
        for i in range(NWARM):
            wt = pp.tile([128, 1024], F32, tag="S", name="warm")
            nc.tensor.matmul(wt[:, 0:128], dummy[:], dummy[:],
                             start=True, stop=True)

        # ---- v16 ones columns (DVE idle early) --------------------------
        for j in range(NJ):
            nc.vector.memset(v16[j][:, :, D:W], 1.0)

        P_tiles = [None] * H

        def emit_score_pair(h, jm):
            """ST chunk jm for head h: 2 matmuls + ACT exp -> P bf16."""
            cc, off = divmod(h, 2)
            off *= D
            if P_tiles[h] is None:
                P_tiles[h] = p_pool.tile([128, mk, N], BF16, tag="P",
                                         name=f"P{h}")
            S = pp.tile([128, N], F32, tag="S", name="S")
            for nn in range(2):
                nc.tensor.matmul(
                    S[:, nn * 512:(nn + 1) * 512],
                    kT[cc][off:off + D, jm * 128:(jm + 1) * 128],
                    qT[cc][off:off + D, nn * 512:(nn + 1) * 512],
                    start=True, stop=True)
            nc.scalar.activation(P_tiles[h][:, jm, :], S[:], AF.Exp,
                                 bias=lm_sb[:, jm:jm + 1])

        dg_flip = [True]

        def emit_pv_head(h):
            """P@v for head h, natural orientation, 8 q-chunk groups.
            One batched reciprocal per head; normalize muls alternate
            DVE/Pool so the O-tile drain keeps up with the PE."""
            O = pp.tile([128, MJ, 128], F32, tag="B", name=f"O{h}")
            rec = rec_pool.tile([128, MJ, 1], F32, tag="rec", name="rec")
            for jn in range(MJ):
                has_diag = jn >= jd
                for jm in range(mk):
                    nc.tensor.matmul(
                        O[:, jn, 0:W],
                        P_tiles[h][:, jm, jn * 128:(jn + 1) * 128],
                        v16[jm][:, h, :],
                        start=(jm == 0),
                        stop=(jm == mk - 1 and not has_diag))
                if has_diag:
                    dg = dg_pool.tile([128, 128], BF16, tag="dg", name="dg")
                    nc.gpsimd.tensor_scalar_mul(dg[:], idb_sb[:],
                                                gm_sb[:, jn, h:h + 1])
                    nc.tensor.matmul(O[:, jn, 0:W], dg[:],
                                     v16[jn][:, h, :],
                                     start=False, stop=True)
            nc.vector.reciprocal(rec[:], O[:, :, D:W])
            with nc.allow_low_precision(reason="fp16 o norm"):
                for jn in range(MJ):
                    nc.vector.tensor_scalar_mul(
                        oN[jn][:, h * D:(h + 1) * D],
                        O[:, jn, 0:D], rec[:, jn, :])

        def emit_pre_tp(jn):
            """Transpose cc 0..CPRE-1 of oN[jn]; DVE drains to oT."""
            tp = pp.tile([128, CPRE, 128], FP16, tag="B", name=f"tp{jn}")
            for cc in range(CPRE):
                nc.tensor.matmul(tp[:, cc, :],
                                 oN[jn][:, cc * 128:(cc + 1) * 128],
                                 id16_sb[:], is_transpose=True,
                                 start=True, stop=True)
            for cc in range(CPRE):
                nc.vector.tensor_copy(oT[cc][:, jn * 128:(jn + 1) * 128],
                                      tp[:, cc, :])

        def emit_pre_proj(jn):
            """Fold the first CPRE proj chunks into ypre[jn].  Uses the
            S-ring: the score stream is finished by the time these run."""
            yps = pp.tile([128, 1024], F32, tag="S", name="yps")
            for sl0, sl1 in ((0, 512), (512, C)):
                for cc in range(CPRE):
                    nc.tensor.matmul(
                        yps[:, sl0:sl1],
                        oT[cc][:, jn * 128:(jn + 1) * 128],
                        wp_sb[cc][:, sl0:sl1],
                        start=(cc == 0), stop=(cc == CPRE - 1))
            with nc.allow_low_precision(reason="fp16 ypre fold"):
                nc.vector.tensor_copy(ypre[jn][:], yps[:, 0:C])

        def emit_tail_tp(jn):
            """Transpose cc CPRE..5 of oN[jn]; ACT drains to oT."""
            tp = pp.tile([128, CH - CPRE, 128], FP16, tag="B",
                         name=f"tt{jn}")
            for i, cc in enumerate(range(CPRE, CH)):
                nc.tensor.matmul(tp[:, i, :],
                                 oN[jn][:, cc * 128:(cc + 1) * 128],
                                 id16_sb[:], is_transpose=True,
                                 start=True, stop=True)
            for i, cc in enumerate(range(CPRE, CH)):
                if jn % 2 == 0:
                    nc.scalar.copy(oT[cc][:, jn * 128:(jn + 1) * 128],
                                   tp[:, i, :])
                else:
                    nc.vector.tensor_copy(
                        oT[cc][:, jn * 128:(jn + 1) * 128], tp[:, i, :])

        def emit_tail_proj(jn):
            """Project cc CPRE..5, add ypre, copy out, DMA."""
            yps = pp.tile([128, 1024], F32, tag="S", name="yts")
            for sl0, sl1 in ((0, 512), (512, C)):
                for i, cc in enumerate(range(CPRE, CH)):
                    nc.tensor.matmul(
                        yps[:, sl0:sl1],
                        oT[cc][:, jn * 128:(jn + 1) * 128],
                        wp_sb[cc][:, sl0:sl1],
                        start=(cc == CPRE), stop=(cc == CH - 1))
            ysb = yout_pool_ref[0].tile([128, C], F32, tag="ysb",
                                        name="ysb")
            with nc.allow_low_precision(reason="f32 add"):
                nc.vector.tensor_add(ysb[:, 0:512], ypre[jn][:, 0:512],
                                     yps[:, 0:512])
                nc.vector.tensor_add(ysb[:, 512:C], ypre[jn][:, 512:C],
                                     yps[:, 512:C])
            nc.sync.dma_start(out=y[jn * 128:(jn + 1) * 128, 0:512],
                              in_=ysb[:, 0:512])
            nc.gpsimd.dma_start(out=y[jn * 128:(jn + 1) * 128, 512:C],
                                in_=ysb[:, 512:C])

        # ================= phase A/B: qkv + scores ======================
        with tc.tile_pool(name="ph1", bufs=1) as ph1:
            xT_sb = [ph1.tile([128, N], FP16, tag=f"xT{kk}", name=f"xs{kk}")
                     for kk in range(CH)]
            wqq_sb = [ph1.tile([128, C], FP16, tag=f"wq{kk}", name=f"wq{kk}")
                      for kk in range(CH)]
            # NOTE: wqq shares ph1's lifetime; P_BUFS budget counts on the
            # post-ph1 release for the oT/ypre/yout pools.
            wqk_sb = [ph1.tile([128, C], FP16, tag=f"wk{kk}", name=f"wk{kk}")
                      for kk in range(CH)]
            wv_sb = [ph1.tile([128, C], FP16, tag=f"wv{kk}", name=f"wv{kk}")
                     for kk in range(CH)]

            # ---- input DMAs (order on each ring == arrival order) -------
            for kk in range(CH):
                deng = nc.sync if kk % 2 == 0 else nc.gpsimd
                deng.dma_start(out=xT_sb[kk][:],
                               in_=xT[kk * 128:(kk + 1) * 128, :])
                deng.dma_start(out=wqq_sb[kk][:],
                               in_=wqkvT[kk * 128:(kk + 1) * 128, 0:C])
                if kk == 1:
                    nc.gpsimd.dma_start(out=cpa_sb[:], in_=cpackA[:, :])
            nc.gpsimd.dma_start(out=eh_sb[:], in_=eheadB[:, :])
            nc.gpsimd.dma_start(out=id16_sb[:], in_=identH[:, :])
            nc.gpsimd.dma_start(out=idb_sb[:], in_=identB[:, :])
            for kk in range(CH):
                deng = nc.sync if kk % 2 == 0 else nc.gpsimd
                deng.dma_start(out=wqk_sb[kk][:],
                               in_=wqkvT[kk * 128:(kk + 1) * 128, C:2 * C])
            for kk in range(CH):
                deng = nc.sync if kk % 2 == 0 else nc.gpsimd
                deng.dma_start(out=wv_sb[kk][:],
                               in_=wqkvT[kk * 128:(kk + 1) * 128, 2 * C:3 * C])
            for cc in range(CH):
                deng = nc.gpsimd if cc % 2 == 0 else nc.sync
                deng.dma_start(out=wp_sb[cc][:],
                               in_=wprojT[cc * 128:(cc + 1) * 128, :])

            def gen_qk(which, cc, split=False):
                """q or k chunk cc: 12 matmuls, then fp16 copy. Yields rows
                after each matmul so score pairs can interleave.  With
                split=True the kk contraction runs as two 3-chunk psum
                groups combined by a DVE add, so the first group only
                depends on the first 3 input DMAs (group deps are hoisted
                to the group's first matmul)."""
                wsrc = wqq_sb if which == "q" else wqk_sb
                dst = (qT if which == "q" else kT)[cc]
                ps = pp.tile([128, N], F32, tag="B", name=f"{which}{cc}")
                if split:
                    ps2 = pp.tile([128, N], F32, tag="S", name=f"{which}{cc}b")
                    for half, (k0, k1) in enumerate(((0, 3), (3, CH))):
                        dstp = ps if half == 0 else ps2
                        for nn in range(2):
                            for kk in range(k0, k1):
                                nc.tensor.matmul(
                                    dstp[:, nn * 512:(nn + 1) * 512],
                                    wsrc[kk][:, cc * 128:(cc + 1) * 128],
                                    xT_sb[kk][:, nn * 512:(nn + 1) * 512],
                                    start=(kk == k0), stop=(kk == k1 - 1))
                                if half == 1 and nn == 1 and kk == k1 - 1:
                                    with nc.allow_low_precision(
                                            reason="fp16 qk combine"):
                                        nc.vector.tensor_copy(dst[:], ps2[:])
                                        nc.vector.tensor_add(dst[:], dst[:],
                                                             ps[:])
                                yield 512
                    return
                for nn in range(2):
                    for kk in range(CH):
                        nc.tensor.matmul(
                            ps[:, nn * 512:(nn + 1) * 512],
                            wsrc[kk][:, cc * 128:(cc + 1) * 128],
                            xT_sb[kk][:, nn * 512:(nn + 1) * 512],
                            start=(kk == 0), stop=(kk == CH - 1))
                        if nn == 1 and kk == CH - 1:
                            nc.vector.tensor_copy(dst[:], ps[:])
                        yield 512

            def gen_v(jn):
                """v chunk jn: 12 matmuls, then strided bf16 copy."""
                ps = pp.tile([128, 1024], F32, tag="B", name=f"vp{jn}")
                for sl0, sl1 in ((0, 512), (512, C)):
                    for kk in range(CH):
                        nc.tensor.matmul(
                            ps[:, sl0:sl1],
                            xT_sb[kk][:, jn * 128:(jn + 1) * 128],
                            wv_sb[kk][:, sl0:sl1],
                            start=(kk == 0), stop=(kk == CH - 1))
                        if sl1 == C and kk == CH - 1:
                            ps3 = ps[:, 0:C].rearrange("p (h d) -> p h d", h=H)
                            nc.vector.tensor_copy(v16[jn][:, :, 0:D], ps3)
                        yield sl1 - sl0

            def gen_gm(prod_pool):
                """diag self-term magnitudes for chunks >= jd.  prod
                reuses the dead wqq weight tiles (last read: q-chunk 5,
                which precedes gm in emission order) via bitcast."""
                prod = []
                nd = MJ - jd
                for cc in range(CH):
                    pr = wqq_sb[cc][:, 0:nd * 128]
                    eng = nc.gpsimd if cc % 2 == 0 else nc.vector
                    with nc.allow_low_precision(reason="fp16 prod"):
                        eng.tensor_mul(pr, qT[cc][:, jd * 128:],
                                       kT[cc][:, jd * 128:])
                    prod.append(pr)
                for jm in range(jd, MJ):
                    gps = pp.tile([128, 1024], F32, tag="B", name="gps")
                    for cc in range(CH):
                        nc.tensor.matmul(
                            gps[:, 0:H],
                            prod[cc][:, (jm - jd) * 128:(jm - jd + 1) * 128],
                            eh_sb[:, cc * H:(cc + 1) * H],
                            start=(cc == 0), stop=(cc == CH - 1))
                        yield H
                    nc.scalar.activation(gm_sb[:, jm, :], gps[:, 0:H], AF.Exp)
                    nc.vector.tensor_scalar_mul(gm_sb[:, jm, :],
                                                gm_sb[:, jm, :],
                                                omp_sb[:, jm:jm + 1])

            def bulk_stream(prod_pool):
                # q0,q1,k0,q2,k1,... : k_cc as early as its wk DMAs allow,
                # so the exp stream starts ~10us in
                yield from gen_qk("q", 0, split=True)
                yield from gen_qk("q", 1)
                yield from gen_qk("k", 0, split=True)
                state["qk_pairs"] = 1
                for cc in range(2, CH):
                    yield from gen_qk("q", cc)
                    yield from gen_qk("k", cc - 1)
                    state["qk_pairs"] = cc
                yield from gen_qk("k", CH - 1)
                state["qk_pairs"] = CH
                yield from gen_gm(prod_pool)
                for jn in range(NJ):
                    yield from gen_v(jn)

            score_list = [(h, jm) for h in range(H) for jm in range(mk)]
            state = {"si": 0, "credit": 0.0, "qk_pairs": 0, "pv": 0}

            def score_eligible():
                if state["si"] >= len(score_list):
                    return False
                h, _ = score_list[state["si"]]
                if (h // 2) >= state["qk_pairs"]:
                    return False
                # P-ring: the exp for head h allocates P slot h % P_BUFS,
                # which frees only when P@v of head h-P_BUFS is done.
                # Emitting the pair earlier jams the S-ring behind it.
                return h < P_BUFS or state["pv"] >= h - P_BUFS + 1

            def pump_scores():
                while state["credit"] >= PACE_ROWS and score_eligible():
                    emit_score_pair(*score_list[state["si"]])
                    state["si"] += 1
                    state["credit"] -= PACE_ROWS
                # no banking: a credit surplus would burst pairs back-to-back
                # and the S-ring (2 slots, ACT-paced) head-of-line blocks
                # everything emitted after them
                state["credit"] = min(state["credit"], 1.2 * PACE_ROWS)

            for rows in bulk_stream(None):
                state["credit"] += rows
                pump_scores()

        # ============ phase C: P@v + pre-projection + tail ==============
        oT_pool = ctx.enter_context(tc.tile_pool(name="oTp", bufs=1))
        ypre_pool = ctx.enter_context(tc.tile_pool(name="ypre", bufs=1))
        yout_pool_ref.append(
            ctx.enter_context(tc.tile_pool(name="yout", bufs=2)))
        oT.extend(oT_pool.tile([128, N], FP16, tag=f"oT{cc}", name=f"oT{cc}")
                  for cc in range(CH))
        ypre.extend(ypre_pool.tile([128, C], FP16, tag=f"yp{j}",
                                   name=f"yp{j}") for j in range(NJ))
        # pv h0..h7 run as soon as their exps/psum allow; deferred score
        # pairs (P-ring gated) are pumped between them.
        for h in range(8):
            emit_pv_head(h)
            state["pv"] = h + 1
            while score_eligible():
                emit_score_pair(*score_list[state["si"]])
                state["si"] += 1
        while score_eligible():
            emit_score_pair(*score_list[state["si"]])
            state["si"] += 1
        # pv h8..h11 execute gated on their exps (~6us apart): distribute
        # the pre-projection units into those gaps, tp/proj pipelined.
        pre_units = []
        pre_units.append(("tp", 0))
        for jn in range(NJ):
            if jn + 1 < NJ:
                pre_units.append(("tp", jn + 1))
            pre_units.append(("proj", jn))
        per_gap = (len(pre_units) + 3) // 4
        ui = 0
        for h in range(8, H):
            emit_pv_head(h)
            state["pv"] = h + 1
            while score_eligible():
                emit_score_pair(*score_list[state["si"]])
                state["si"] += 1
            for _ in range(per_gap):
                if ui < len(pre_units):
                    kind, jn = pre_units[ui]
                    (emit_pre_tp if kind == "tp" else emit_pre_proj)(jn)
                    ui += 1
        while ui < len(pre_units):
            kind, jn = pre_units[ui]
            (emit_pre_tp if kind == "tp" else emit_pre_proj)(jn)
            ui += 1
        emit_tail_tp(0)
        emit_tail_tp(1)
        for jn in range(NJ):
            emit_tail_proj(jn)
            if jn + 2 < NJ:
                emit_tail_tp(jn + 2)

    nc.finalize()
    return nc


_NC_CACHE = {}


def _get_nc(mk: int = MJ, jd: int = 0):
    if (mk, jd) not in _NC_CACHE:
        _NC_CACHE[(mk, jd)] = build_nc(mk, jd)
    return _NC_CACHE[(mk, jd)]


def _to_bf16(a):
    import ml_dtypes
    return np.asarray(a, np.float32).astype(ml_dtypes.bfloat16)


def _host_inputs(x, policy, w_qkv, w_proj, b_proj):
    """Shard + permute (kept tokens first) + layout transforms.

    Returns (in_maps, perms, mk, jd)."""
    wqkv_s = np.array(w_qkv, dtype=np.float32, copy=True)
    wqkv_s[0:C] *= np.float32(SCALE)
    wqkvT = np.ascontiguousarray(wqkv_s.T).astype(np.float16)   # [C, 3C]
    wprojT = np.ascontiguousarray(
        np.asarray(w_proj, np.float32).T).astype(np.float16)

    E = np.zeros((C, H), np.float32)
    for c in range(C):
        E[c, c // D] = 1.0
    eheadB = np.ascontiguousarray(
        E.reshape(CH, 128, H).transpose(1, 0, 2).reshape(128, CH * H)
    ).astype(np.float16)
    ident = np.eye(128, dtype=np.float32)
    identH = ident.astype(np.float16)
    identB = _to_bf16(ident)

    in_maps = []
    perms = []
    mk = 1
    jd = MJ - 1
    for b in range(B):
        pol = np.asarray(policy[b], np.float32).reshape(N)
        kept = np.nonzero(pol > 0.5)[0]
        drop = np.nonzero(pol <= 0.5)[0]
        perm = np.concatenate([kept, drop])
        perms.append(perm)
        mk = max(mk, (len(kept) + 127) // 128)
        jd = min(jd, len(kept) // 128)

        xb = np.asarray(x[b], np.float32)[perm, :]          # permuted tokens
        xTb = np.ascontiguousarray(xb.T).astype(np.float16)  # [C, N]
        polp = pol[perm]
        lm = np.where(polp > 0.5, 0.0, NEG).astype(np.float32)
        lm = np.ascontiguousarray(lm.reshape(MJ, 128).T)    # [128, MJ]
        om = np.ascontiguousarray((1.0 - polp).reshape(MJ, 128).T)
        cpackA = np.ascontiguousarray(np.concatenate(
            [lm, om.astype(np.float32)], axis=1))
        in_maps.append({
            "xT": xTb, "wqkvT": wqkvT, "wprojT": wprojT,
            "cpackA": cpackA, "eheadB": eheadB,
            "identH": identH, "identB": identB,
        })
    return in_maps, perms, mk, jd


def kernel(x, policy, w_qkv, w_proj, b_proj):
    from concourse.bass_utils import run_bass_kernel_spmd

    x = np.asarray(x, np.float32)
    policy = np.asarray(policy, np.float32)
    w_qkv = np.asarray(w_qkv, np.float32)
    w_proj = np.asarray(w_proj, np.float32)
    b_proj = np.asarray(b_proj, np.float32)
    in_maps, perms, mk, jd = _host_inputs(x, policy, w_qkv, w_proj, b_proj)
    nc = _get_nc(mk, jd)
    res = run_bass_kernel_spmd(nc, in_maps, list(range(B)))
    out = np.empty((B, N, C), np.float32)
    bp = np.asarray(b_proj, np.float32).reshape(1, C)
    for b in range(B):
        out[b][perms[b]] = res.results[b]["y"] + bp
    return out


# revision 5
# speedup vs baseline: 1.1103x; 1.0022x over previous
"""Trainium2 Bass kernel for masked (sparse) multi-head attention, v2.1.

Reference (per batch): qkv = x @ w_qkv.T; q *= D**-0.5; s = q@k.T per head;
e = exp(s) * ap  (ap = key policy, self-attend always allowed);
attn = e / sum_m e; y = (attn @ v) @ w_proj.T + b_proj (b_proj on host).

Sharding: data parallel, batch b -> core b (B == n_cores == 8).

Design (CoreSim cost model: a matmul costs out-free-size rows x 0.4167ns,
independent of K and M; ACT activations cost ~0.833ns/elem/partition):
  - P@v in NATURAL orientation (queries on psum partitions, head dim on
    the free axis): 12h*8jn*6jm*65 rows instead of 12h*6jm*1024.  P =
    exp(ST) feeds it directly as lhsT; v blocks carry a ones column at
    col D so each (head, qchunk) psum accumulates its softmax
    denominator; normalization is a per-partition DVE scalar multiply.
  - normalized o (fp16) is PE-transposed per [128,128] block; projection
    contracts K=128 stacked c-chunks (8jn*6cc*768 rows, vs 12h*8*768 at
    K=64 before).  The projection is SPLIT: cc 0..3 (heads 0..7) run
    while later heads' exps still stream, folded to SBUF; cc 4..5 +
    a DVE/Pool add finish after the last exp.
  - P / v / diag are bf16 (exp(s) reaches ~1e6; fp16 would overflow);
    qT/kT and the transpose/projection pipeline are fp16.
  - single psum pool, tags S and B ([128,1024] x 2 bufs each = 8 banks).
    S ring: warmup, score S tiles, proj y tiles.  B ring: qk psums,
    v psums, gm psums, P@v O tiles, transpose tiles.
  - emission merges the bulk stream (qk, v, gm) with exp-paced score
    pairs, P@v heads, and pre-projection groups so the ACT engine (72
    exps ~ 75us) is the only near-critical chain and PE never
    head-of-line blocks.
  - separate q-half / k-half weight tiles (a shared tile's second DMA
    writer falsely serialized the first matmul); ~15 warmup matmuls on a
    memset tile cover the PE p-state ramp while the first DMAs land.
"""

import sys

import numpy as np

sys.path.insert(0, "/opt/trn_rl_repo")

from contextlib import ExitStack

import concourse.bass as bass
import concourse.tile as tile
from concourse import mybir
from concourse.bacc import Bacc

F32 = mybir.dt.float32
F32R = mybir.dt.float32r
BF16 = mybir.dt.bfloat16
FP16 = mybir.dt.float16
AF = mybir.ActivationFunctionType

B, N, C, H = 8, 1024, 768, 12
D = C // H             # 64
SCALE = D ** -0.5
CH = C // 128           # 6 c-chunks (2 heads each)
NJ = N // 128           # 8 n-chunks
MJ = N // 128           # 8 m-chunks
NEG = -10000.0          # exp(s + NEG) == 0.0 for any realistic s
W = D + 1               # per-head v block width; ones col at D
NWARM = 16              # PE p-state warmup matmuls
PACE_ROWS = 1900        # bulk rows between score pairs; a pair itself is
                        # 1024 rows, so S-tile spacing ~= ACT exp cadence
P_BUFS = 9              # P-tile backlog (heads in flight)
CPRE = 4                # proj cc-chunks folded early (heads 0..2*CPRE-1)


def build_nc(mk: int, jd: int) -> bass.Bass:
    """mk = chunks holding all kept tokens; jd = first chunk with any
    dropped token (diag machinery only needed for chunks >= jd)."""
    nc = Bacc()

    xT = nc.declare_dram_parameter("xT", [C, N], FP16, isOutput=False)
    wqkvT = nc.declare_dram_parameter("wqkvT", [C, 3 * C], FP16, isOutput=False)
    wprojT = nc.declare_dram_parameter("wprojT", [C, C], FP16, isOutput=False)
    cpackA = nc.declare_dram_parameter("cpackA", [128, 2 * MJ], F32,
                                       isOutput=False)
    eheadB = nc.declare_dram_parameter("eheadB", [128, CH * H], FP16,
                                       isOutput=False)
    identH = nc.declare_dram_parameter("identH", [128, 128], FP16,
                                       isOutput=False)
    identB = nc.declare_dram_parameter("identB", [128, 128], BF16,
                                       isOutput=False)
    y = nc.declare_dram_parameter("y", [N, C], F32, isOutput=True)

    with ExitStack() as ctx:
        tc = ctx.enter_context(tile.TileContext(nc))

        consts = ctx.enter_context(tc.tile_pool(name="consts", bufs=1))
        qk_pool = ctx.enter_context(tc.tile_pool(name="qk", bufs=1))
        v_pool = ctx.enter_context(tc.tile_pool(name="v", bufs=1))
        wp_pool = ctx.enter_context(tc.tile_pool(name="wpp", bufs=1))
        p_pool = ctx.enter_context(tc.tile_pool(name="pp", bufs=P_BUFS))
        oN_pool = ctx.enter_context(tc.tile_pool(name="oN", bufs=1))
        rec_pool = ctx.enter_context(tc.tile_pool(name="rec", bufs=2))
        dg_pool = ctx.enter_context(tc.tile_pool(name="dg", bufs=4))
        pp = ctx.enter_context(tc.tile_pool(name="psum", bufs=2, space="PSUM"))

        # ---- constants --------------------------------------------------
        cpa_sb = consts.tile([128, 2 * MJ], F32, tag="cpa", name="cpa")
        lm_sb = cpa_sb[:, 0:MJ]
        omp_sb = cpa_sb[:, MJ:2 * MJ]
        eh_sb = consts.tile([128, CH * H], FP16, tag="eh", name="eh")
        id16_sb = consts.tile([128, 128], FP16, tag="id16", name="id16")
        idb_sb = consts.tile([128, 128], BF16, tag="idb", name="idb")
        dummy = consts.tile([128, 128], FP16, tag="dummy", name="dummy")
        gm_sb = consts.tile([128, MJ, H], F32, tag="gm", name="gm")

        # persistent activation tiles
        qT = [qk_pool.tile([128, N], FP16, tag=f"qT{cc}", name=f"qT{cc}")
              for cc in range(CH)]
        kT = [qk_pool.tile([128, N], FP16, tag=f"kT{cc}", name=f"kT{cc}")
              for cc in range(CH)]
        v16 = [v_pool.tile([128, H, W], BF16, tag=f"v{j}", name=f"v{j}")
               for j in range(NJ)]
        wp_sb = [wp_pool.tile([128, C], FP16, tag=f"wp{cc}", name=f"wp{cc}")
                 for cc in range(CH)]
        oN = [oN_pool.tile([128, C], FP16, tag=f"oN{j}", name=f"oN{j}")
              for j in range(NJ)]
        # oT / ypre / yout pools are created after the ph1 input pool
        # closes so their SBUF reuses the freed xT/w space (stack alloc).
        oT = []
        ypre = []
        yout_pool_ref = []

        # ---- warmup: cover the PE p-state ramp before DMAs land ---------
        nc.vector.memset(dummy[:], 0.0)
        for i in range(NWARM):
            wt = pp.tile([128, 1024], F32, tag="S", name="warm")
            nc.tensor.matmul(wt[:, 0:128], dummy[:], dummy[:],
                             start=True, stop=True)

        # ---- v16 ones columns (DVE idle early) --------------------------
        for j in range(NJ):
            nc.vector.memset(v16[j][:, :, D:W], 1.0)

        P_tiles = [None] * H

        def emit_score_pair(h, jm):
            """ST chunk jm for head h: 2 matmuls + ACT exp -> P bf16."""
            cc, off = divmod(h, 2)
            off *= D
            if P_tiles[h] is None:
                P_tiles[h] = p_pool.tile([128, mk, N], BF16, tag="P",
                                         name=f"P{h}")
            S = pp.tile([128, N], F32, tag="S", name="S")
            for nn in range(2):
                nc.tensor.matmul(
                    S[:, nn * 512:(nn + 1) * 512],
                    kT[cc][off:off + D, jm * 128:(jm + 1) * 128],
                    qT[cc][off:off + D, nn * 512:(nn + 1) * 512],
                    start=True, stop=True)
            nc.scalar.activation(P_tiles[h][:, jm, :], S[:], AF.Exp,
                                 bias=lm_sb[:, jm:jm + 1])

        dg_flip = [True]

        def emit_pv_head(h):
            """P@v for head h, natural orientation, 8 q-chunk groups.
            One batched reciprocal per head; normalize muls alternate
            DVE/Pool so the O-tile drain keeps up with the PE."""
            O = pp.tile([128, MJ, 128], F32, tag="B", name=f"O{h}")
            rec = rec_pool.tile([128, MJ, 1], F32, tag="rec", name="rec")
            for jn in range(MJ):
                has_diag = jn >= jd
                for jm in range(mk):
                    nc.tensor.matmul(
                        O[:, jn, 0:W],
                        P_tiles[h][:, jm, jn * 128:(jn + 1) * 128],
                        v16[jm][:, h, :],
                        start=(jm == 0),
                        stop=(jm == mk - 1 and not has_diag))
                if has_diag:
                    dg = dg_pool.tile([128, 128], BF16, tag="dg", name="dg")
                    nc.gpsimd.tensor_scalar_mul(dg[:], idb_sb[:],
                                                gm_sb[:, jn, h:h + 1])
                    nc.tensor.matmul(O[:, jn, 0:W], dg[:],
                                     v16[jn][:, h, :],
                                     start=False, stop=True)
            nc.vector.reciprocal(rec[:], O[:, :, D:W])
            with nc.allow_low_precision(reason="fp16 o norm"):
                for jn in range(MJ):
                    nc.vector.tensor_scalar_mul(
                        oN[jn][:, h * D:(h + 1) * D],
                        O[:, jn, 0:D], rec[:, jn, :])

        def emit_pre_tp(jn):
            """Transpose cc 0..CPRE-1 of oN[jn]; DVE drains to oT."""
            tp = pp.tile([128, CPRE, 128], FP16, tag="B", name=f"tp{jn}")
            for cc in range(CPRE):
                nc.tensor.matmul(tp[:, cc, :],
                                 oN[jn][:, cc * 128:(cc + 1) * 128],
                                 id16_sb[:], is_transpose=True,
                                 start=True, stop=True)
            for cc in range(CPRE):
                if jn >= 4:
                    nc.scalar.copy(oT[cc][:, jn * 128:(jn + 1) * 128],
                                   tp[:, cc, :])
                else:
                    nc.vector.tensor_copy(
                        oT[cc][:, jn * 128:(jn + 1) * 128], tp[:, cc, :])

        def emit_pre_proj(jn):
            """Fold the first CPRE proj chunks into ypre[jn].  Uses the
            S-ring: the score stream is finished by the time these run."""
            yps = pp.tile([128, 1024], F32, tag="S", name="yps")
            for sl0, sl1 in ((0, 512), (512, C)):
                for cc in range(CPRE):
                    nc.tensor.matmul(
                        yps[:, sl0:sl1],
                        oT[cc][:, jn * 128:(jn + 1) * 128],
                        wp_sb[cc][:, sl0:sl1],
                        start=(cc == 0), stop=(cc == CPRE - 1))
            with nc.allow_low_precision(reason="fp16 ypre fold"):
                nc.vector.tensor_copy(ypre[jn][:], yps[:, 0:C])

        def emit_tail_tp(jn):
            """Transpose cc CPRE..5 of oN[jn]; ACT drains to oT."""
            tp = pp.tile([128, CH - CPRE, 128], FP16, tag="B",
                         name=f"tt{jn}")
            for i, cc in enumerate(range(CPRE, CH)):
                nc.tensor.matmul(tp[:, i, :],
                                 oN[jn][:, cc * 128:(cc + 1) * 128],
                                 id16_sb[:], is_transpose=True,
                                 start=True, stop=True)
            for i, cc in enumerate(range(CPRE, CH)):
                nc.scalar.copy(oT[cc][:, jn * 128:(jn + 1) * 128],
                               tp[:, i, :])

        def emit_tail_proj(jn):
            """Project cc CPRE..5, add ypre, copy out, DMA."""
            yps = pp.tile([128, 1024], F32, tag="S", name="yts")
            for sl0, sl1 in ((0, 512), (512, C)):
                for i, cc in enumerate(range(CPRE, CH)):
                    nc.tensor.matmul(
                        yps[:, sl0:sl1],
                        oT[cc][:, jn * 128:(jn + 1) * 128],
                        wp_sb[cc][:, sl0:sl1],
                        start=(cc == CPRE), stop=(cc == CH - 1))
            ysb = yout_pool_ref[0].tile([128, C], F32, tag="ysb",
                                        name="ysb")
            with nc.allow_low_precision(reason="f32 add"):
                nc.vector.tensor_add(ysb[:, 0:512], ypre[jn][:, 0:512],
                                     yps[:, 0:512])
                nc.vector.tensor_add(ysb[:, 512:C], ypre[jn][:, 512:C],
                                     yps[:, 512:C])
            nc.sync.dma_start(out=y[jn * 128:(jn + 1) * 128, 0:512],
                              in_=ysb[:, 0:512])
            nc.gpsimd.dma_start(out=y[jn * 128:(jn + 1) * 128, 512:C],
                                in_=ysb[:, 512:C])

        # ================= phase A/B: qkv + scores ======================
        with tc.tile_pool(name="ph1", bufs=1) as ph1:
            xT_sb = [ph1.tile([128, N], FP16, tag=f"xT{kk}", name=f"xs{kk}")
                     for kk in range(CH)]
            wqq_sb = [ph1.tile([128, C], FP16, tag=f"wq{kk}", name=f"wq{kk}")
                      for kk in range(CH)]
            # NOTE: wqq shares ph1's lifetime; P_BUFS budget counts on the
            # post-ph1 release for the oT/ypre/yout pools.
            wqk_sb = [ph1.tile([128, C], FP16, tag=f"wk{kk}", name=f"wk{kk}")
                      for kk in range(CH)]
            wv_sb = [ph1.tile([128, C], FP16, tag=f"wv{kk}", name=f"wv{kk}")
                     for kk in range(CH)]

            # ---- input DMAs (order on each ring == arrival order) -------
            for kk in range(CH):
                deng = nc.sync if kk % 2 == 0 else nc.gpsimd
                deng.dma_start(out=xT_sb[kk][:],
                               in_=xT[kk * 128:(kk + 1) * 128, :])
                deng.dma_start(out=wqq_sb[kk][:],
                               in_=wqkvT[kk * 128:(kk + 1) * 128, 0:C])
                if kk == 1:
                    nc.gpsimd.dma_start(out=cpa_sb[:], in_=cpackA[:, :])
            nc.gpsimd.dma_start(out=eh_sb[:], in_=eheadB[:, :])
            nc.gpsimd.dma_start(out=id16_sb[:], in_=identH[:, :])
            nc.gpsimd.dma_start(out=idb_sb[:], in_=identB[:, :])
            for kk in range(CH):
                deng = nc.sync if kk % 2 == 0 else nc.gpsimd
                deng.dma_start(out=wqk_sb[kk][:],
                               in_=wqkvT[kk * 128:(kk + 1) * 128, C:2 * C])
            for kk in range(CH):
                deng = nc.sync if kk % 2 == 0 else nc.gpsimd
                deng.dma_start(out=wv_sb[kk][:],
                               in_=wqkvT[kk * 128:(kk + 1) * 128, 2 * C:3 * C])
            for cc in range(CH):
                deng = nc.gpsimd if cc % 2 == 0 else nc.sync
                deng.dma_start(out=wp_sb[cc][:],
                               in_=wprojT[cc * 128:(cc + 1) * 128, :])

            def gen_qk(which, cc, split=False):
                """q or k chunk cc: 12 matmuls, then fp16 copy. Yields rows
                after each matmul so score pairs can interleave.  With
                split=True the kk contraction runs as two 3-chunk psum
                groups combined by a DVE add, so the first group only
                depends on the first 3 input DMAs (group deps are hoisted
                to the group's first matmul)."""
                wsrc = wqq_sb if which == "q" else wqk_sb
                dst = (qT if which == "q" else kT)[cc]
                ps = pp.tile([128, N], F32, tag="B", name=f"{which}{cc}")
                if split:
                    ps2 = pp.tile([128, N], F32, tag="S", name=f"{which}{cc}b")
                    for half, (k0, k1) in enumerate(((0, 3), (3, CH))):
                        dstp = ps if half == 0 else ps2
                        for nn in range(2):
                            for kk in range(k0, k1):
                                nc.tensor.matmul(
                                    dstp[:, nn * 512:(nn + 1) * 512],
                                    wsrc[kk][:, cc * 128:(cc + 1) * 128],
                                    xT_sb[kk][:, nn * 512:(nn + 1) * 512],
                                    start=(kk == k0), stop=(kk == k1 - 1))
                                if half == 1 and nn == 1 and kk == k1 - 1:
                                    with nc.allow_low_precision(
                                            reason="fp16 qk combine"):
                                        nc.vector.tensor_copy(dst[:], ps2[:])
                                        nc.vector.tensor_add(dst[:], dst[:],
                                                             ps[:])
                                yield 512
                    return
                for nn in range(2):
                    for kk in range(CH):
                        nc.tensor.matmul(
                            ps[:, nn * 512:(nn + 1) * 512],
                            wsrc[kk][:, cc * 128:(cc + 1) * 128],
                            xT_sb[kk][:, nn * 512:(nn + 1) * 512],
                            start=(kk == 0), stop=(kk == CH - 1))
                        if nn == 1 and kk == CH - 1:
                            nc.vector.tensor_copy(dst[:], ps[:])
                        yield 512

            def gen_v(jn):
                """v chunk jn: 12 matmuls, then strided bf16 copy."""
                ps = pp.tile([128, 1024], F32, tag="B", name=f"vp{jn}")
                for sl0, sl1 in ((0, 512), (512, C)):
                    for kk in range(CH):
                        nc.tensor.matmul(
                            ps[:, sl0:sl1],
                            xT_sb[kk][:, jn * 128:(jn + 1) * 128],
                            wv_sb[kk][:, sl0:sl1],
                            start=(kk == 0), stop=(kk == CH - 1))
                        if sl1 == C and kk == CH - 1:
                            ps3 = ps[:, 0:C].rearrange("p (h d) -> p h d", h=H)
                            nc.vector.tensor_copy(v16[jn][:, :, 0:D], ps3)
                        yield sl1 - sl0

            def gen_gm(prod_pool):
                """diag self-term magnitudes for chunks >= jd.  prod
                reuses the dead wqq weight tiles (last read: q-chunk 5,
                which precedes gm in emission order) via bitcast."""
                prod = []
                nd = MJ - jd
                for cc in range(CH):
                    pr = wqq_sb[cc][:, 0:nd * 128]
                    eng = nc.gpsimd if cc % 2 == 0 else nc.vector
                    with nc.allow_low_precision(reason="fp16 prod"):
                        eng.tensor_mul(pr, qT[cc][:, jd * 128:],
                                       kT[cc][:, jd * 128:])
                    prod.append(pr)
                for jm in range(jd, MJ):
                    gps = pp.tile([128, 1024], F32, tag="B", name="gps")
                    for cc in range(CH):
                        nc.tensor.matmul(
                            gps[:, 0:H],
                            prod[cc][:, (jm - jd) * 128:(jm - jd + 1) * 128],
                            eh_sb[:, cc * H:(cc + 1) * H],
                            start=(cc == 0), stop=(cc == CH - 1))
                        yield H
                    nc.scalar.activation(gm_sb[:, jm, :], gps[:, 0:H], AF.Exp)
                    nc.vector.tensor_scalar_mul(gm_sb[:, jm, :],
                                                gm_sb[:, jm, :],
                                                omp_sb[:, jm:jm + 1])

            def bulk_stream(prod_pool):
                # q0,q1,k0,q2,k1,... : k_cc as early as its wk DMAs allow,
                # so the exp stream starts ~10us in
                yield from gen_qk("q", 0, split=True)
                yield from gen_qk("q", 1)
                yield from gen_qk("k", 0, split=True)
                state["qk_pairs"] = 1
                for cc in range(2, CH):
                    yield from gen_qk("q", cc)
                    yield from gen_qk("k", cc - 1)
                    state["qk_pairs"] = cc
                yield from gen_qk("k", CH - 1)
                state["qk_pairs"] = CH
                yield from gen_gm(prod_pool)
                for jn in range(NJ):
                    yield from gen_v(jn)

            score_list = [(h, jm) for h in range(H) for jm in range(mk)]
            state = {"si": 0, "credit": 0.0, "qk_pairs": 0, "pv": 0}

            def score_eligible():
                if state["si"] >= len(score_list):
                    return False
                h, _ = score_list[state["si"]]
                if (h // 2) >= state["qk_pairs"]:
                    return False
                # P-ring: the exp for head h allocates P slot h % P_BUFS,
                # which frees only when P@v of head h-P_BUFS is done.
                # Emitting the pair earlier jams the S-ring behind it.
                return h < P_BUFS or state["pv"] >= h - P_BUFS + 1

            def pump_scores():
                while state["credit"] >= PACE_ROWS and score_eligible():
                    emit_score_pair(*score_list[state["si"]])
                    state["si"] += 1
                    state["credit"] -= PACE_ROWS
                # no banking: a credit surplus would burst pairs back-to-back
                # and the S-ring (2 slots, ACT-paced) head-of-line blocks
                # everything emitted after them
                state["credit"] = min(state["credit"], 1.2 * PACE_ROWS)

            for rows in bulk_stream(None):
                state["credit"] += rows
                pump_scores()

        # ============ phase C: P@v + pre-projection + tail ==============
        oT_pool = ctx.enter_context(tc.tile_pool(name="oTp", bufs=1))
        ypre_pool = ctx.enter_context(tc.tile_pool(name="ypre", bufs=1))
        yout_pool_ref.append(
            ctx.enter_context(tc.tile_pool(name="yout", bufs=2)))
        oT.extend(oT_pool.tile([128, N], FP16, tag=f"oT{cc}", name=f"oT{cc}")
                  for cc in range(CH))
        ypre.extend(ypre_pool.tile([128, C], FP16, tag=f"yp{j}",
                                   name=f"yp{j}") for j in range(NJ))
        # pv h0..h7 run as soon as their exps/psum allow; deferred score
        # pairs (P-ring gated) are pumped between them.
        for h in range(8):
            emit_pv_head(h)
            state["pv"] = h + 1
            while score_eligible():
                emit_score_pair(*score_list[state["si"]])
                state["si"] += 1
        while score_eligible():
            emit_score_pair(*score_list[state["si"]])
            state["si"] += 1
        # pv h8..h11 execute gated on their exps (~6us apart): distribute
        # the pre-projection units into those gaps, tp/proj pipelined.
        pre_units = []
        pre_units.append(("tp", 0))
        for jn in range(NJ):
            if jn + 1 < NJ:
                pre_units.append(("tp", jn + 1))
            pre_units.append(("proj", jn))
        per_gap = (len(pre_units) + 3) // 4
        ui = 0
        for h in range(8, H):
            emit_pv_head(h)
            state["pv"] = h + 1
            while score_eligible():
                emit_score_pair(*score_list[state["si"]])
                state["si"] += 1
            for _ in range(per_gap):
                if ui < len(pre_units):
                    kind, jn = pre_units[ui]
                    (emit_pre_tp if kind == "tp" else emit_pre_proj)(jn)
                    ui += 1
        while ui < len(pre_units):
            kind, jn = pre_units[ui]
            (emit_pre_tp if kind == "tp" else emit_pre_proj)(jn)
            ui += 1
        emit_tail_tp(0)
        emit_tail_tp(1)
        for jn in range(NJ):
            emit_tail_proj(jn)
            if jn + 2 < NJ:
                emit_tail_tp(jn + 2)

    nc.finalize()
    return nc


_NC_CACHE = {}


def _get_nc(mk: int = MJ, jd: int = 0):
    if (mk, jd) not in _NC_CACHE:
        _NC_CACHE[(mk, jd)] = build_nc(mk, jd)
    return _NC_CACHE[(mk, jd)]


def _to_bf16(a):
    import ml_dtypes
    return np.asarray(a, np.float32).astype(ml_dtypes.bfloat16)


def _host_inputs(x, policy, w_qkv, w_proj, b_proj):
    """Shard + permute (kept tokens first) + layout transforms.

    Returns (in_maps, perms, mk, jd)."""
    wqkv_s = np.array(w_qkv, dtype=np.float32, copy=True)
    wqkv_s[0:C] *= np.float32(SCALE)
    wqkvT = np.ascontiguousarray(wqkv_s.T).astype(np.float16)   # [C, 3C]
    wprojT = np.ascontiguousarray(
        np.asarray(w_proj, np.float32).T).astype(np.float16)

    E = np.zeros((C, H), np.float32)
    for c in range(C):
        E[c, c // D] = 1.0
    eheadB = np.ascontiguousarray(
        E.reshape(CH, 128, H).transpose(1, 0, 2).reshape(128, CH * H)
    ).astype(np.float16)
    ident = np.eye(128, dtype=np.float32)
    identH = ident.astype(np.float16)
    identB = _to_bf16(ident)

    in_maps = []
    perms = []
    mk = 1
    jd = MJ - 1
    for b in range(B):
        pol = np.asarray(policy[b], np.float32).reshape(N)
        kept = np.nonzero(pol > 0.5)[0]
        drop = np.nonzero(pol <= 0.5)[0]
        perm = np.concatenate([kept, drop])
        perms.append(perm)
        mk = max(mk, (len(kept) + 127) // 128)
        jd = min(jd, len(kept) // 128)

        xb = np.asarray(x[b], np.float32)[perm, :]          # permuted tokens
        xTb = np.ascontiguousarray(xb.T).astype(np.float16)  # [C, N]
        polp = pol[perm]
        lm = np.where(polp > 0.5, 0.0, NEG).astype(np.float32)
        lm = np.ascontiguousarray(lm.reshape(MJ, 128).T)    # [128, MJ]
        om = np.ascontiguousarray((1.0 - polp).reshape(MJ, 128).T)
        cpackA = np.ascontiguousarray(np.concatenate(
            [lm, om.astype(np.float32)], axis=1))
        in_maps.append({
            "xT": xTb, "wqkvT": wqkvT, "wprojT": wprojT,
            "cpackA": cpackA, "eheadB": eheadB,
            "identH": identH, "identB": identB,
        })
    return in_maps, perms, mk, jd


def kernel(x, policy, w_qkv, w_proj, b_proj):
    from concourse.bass_utils import run_bass_kernel_spmd

    x = np.asarray(x, np.float32)
    policy = np.asarray(policy, np.float32)
    w_qkv = np.asarray(w_qkv, np.float32)
    w_proj = np.asarray(w_proj, np.float32)
    b_proj = np.asarray(b_proj, np.float32)
    in_maps, perms, mk, jd = _host_inputs(x, policy, w_qkv, w_proj, b_proj)
    nc = _get_nc(mk, jd)
    res = run_bass_kernel_spmd(nc, in_maps, list(range(B)))
    out = np.empty((B, N, C), np.float32)
    bp = np.asarray(b_proj, np.float32).reshape(1, C)
    for b in range(B):
        out[b][perms[b]] = res.results[b]["y"] + bp
    return out
